# revision 47
# baseline (speedup 1.0000x reference)
"""GAT (graph attention) layer on 8 Trainium2 NeuronCores.

Reference math (per head h):
    Wh = x @ W[h];  f_src = Wh @ a_src[h];  f_dst = Wh @ a_dst[h]
    e[i,j] = leaky_relu(f_src[i] + f_dst[j], alpha)
    att = softmax(where(adj>0, e, -9e15), axis=j)
    out[:, h*D:(h+1)*D] = att @ Wh

Exact identity used (p_i = exp(alpha*f_src_i) cancels in the softmax):
    exp(leaky_relu(s)) = exp(alpha*s) * max(1, exp((1-alpha)*s))
    nhat[j,i] = adj[j,i] * max(q_j, u_i * v_j)
      with u_i = exp((1-alpha)*f_src_i), v_j = exp(f_dst_j),
           q_j = exp(alpha*f_dst_j)
    out_h[i,:] = (sum_j nhat[j,i]*Wh[j,:]) / (sum_j nhat[j,i])

Inner loop per (j-block, head-pair): two dual-scalar tensor_scalar ops
    t_h = max(U_h * v_j, q_j)        (DVE 4x mode, per-partition scalars)
then the mask multiply t_h * adj is split by columns between DVE
(tensor_tensor at 2x) and Pool (tensor_tensor, runs in parallel), and
accumulated into PSUM by the usual matmuls.

Phase B runs as two head-pair sweeps of 4 PSUM banks each so sweep 1
overlaps phase A's Wh production (which needs the other banks), and
each pair's phase C overlaps the other sweep. DMA issue order keeps
the small weight transfers ahead of the bulk xT streams; q/v/U are
produced early so the sweeps start as soon as possible.

Distribution: destination rows i sharded 1024/core; host passes adj.T
column-slices cast to bf16 (exact for a 0/1 mask) and x pre-transposed
in bf16, so no device-side transposes or converts are needed.
"""

import os
import numpy as np
from contextlib import ExitStack

import concourse.bass as bass
import concourse.tile as tile
from concourse import bacc, mybir
from concourse.bass_utils import run_bass_kernel_spmd
from concourse.masks import make_identity

N = 8192
DIN = 256
DOUT = 64
H = 4
NCORES = 8
SL = N // NCORES          # 1024 i's per core
NB = N // 128             # 64 j-blocks
SB = SL // 128            # 8 i-blocks per core
ALPHA = 0.2
W4C = H * DOUT            # 256
WAUG = H * (DOUT + 1)     # 260

f32 = mybir.dt.float32
bf16 = mybir.dt.bfloat16
EXP = mybir.ActivationFunctionType.Exp
COPY = mybir.ActivationFunctionType.Copy
MAX = mybir.AluOpType.max
MULT = mybir.AluOpType.mult

_CACHE = {}

NBLIM = int(os.environ.get("NBLIM", str(NB)))
BBUFS = int(os.environ.get("BBUFS", "4"))
CUT = int(os.environ.get("CUT", "684"))     # DVE/Pool mask column split


def _build_module():
    nc = bacc.Bacc("TRN2", target_bir_lowering=False, debug=False,
                   num_devices=NCORES)

    xt_d = nc.dram_tensor("xT_full", [DIN, N], bf16, kind="ExternalInput").ap()
    xst_d = nc.dram_tensor("xT_slice", [DIN, SL], bf16, kind="ExternalInput").ap()
    wf_d = nc.dram_tensor("wf_all", [128, 2, W4C + 8], bf16,
                          kind="ExternalInput").ap()
    ehot_d = nc.dram_tensor("ehot_const", [4, H * 128], bf16,
                            kind="ExternalInput").ap()
    adjT_d = nc.dram_tensor("adjT_slice", [N, SL], bf16, kind="ExternalInput").ap()
    out_d = nc.dram_tensor("out_slice", [SL, H * DOUT], bf16,
                       kind="ExternalOutput").ap()

    with tile.TileContext(nc) as tc, ExitStack() as ctx:
        # ---------------- persistent tiles ----------------
        persist = ctx.enter_context(tc.tile_pool(name="persist", bufs=1))
        what_sb = persist.tile([128, NB, WAUG], bf16)   # Wh_aug per j-block
        u_sb = persist.tile([128, H, SL], bf16)         # exp((1-a)*f_src) bcast
        fall_sb = persist.tile([128, NB, 2 * H], f32)   # [f_src(4) | f_dst(4)]
        q_sb = persist.tile([128, H, NB], f32)          # exp(alpha*f_dst)
        v_sb = persist.tile([128, H, NB], f32)          # exp(f_dst)
        ps_sb = persist.tile([DOUT + 1, H, SL], f32)    # phase C staging
        ident_sb = persist.tile([128, 128], f32)
        make_identity(nc, ident_sb[:])
        bpool = ctx.enter_context(tc.tile_pool(name="bphase", bufs=BBUFS))
        adjpre = {}

        # ======================= PHASE A =======================
        with ExitStack() as actx:
            a1pool = actx.enter_context(tc.tile_pool(name="aphase1", bufs=1))

            # --- fused weight matrix [W4 | wtilde] (host-prepared) ---
            wf_b = a1pool.tile([128, 2, W4C + 8], bf16)
            nc.sync.dma_start(wf_b[:], wf_d)

            # --- own-slice f_src -> broadcast -> U  (early: unblocks B) ---
            with ExitStack() as sctx:
                fpsum = sctx.enter_context(
                    tc.tile_pool(name="apsum_f", bufs=3, space="PSUM"))
                xst_sb = a1pool.tile([128, 2, SL], bf16)
                for hf in range(2):
                    nc.sync.dma_start(
                        xst_sb[:, :, hf * 512:(hf + 1) * 512],
                        xst_d.rearrange("(c p) n -> p c n", p=128)
                            [:, :, hf * 512:(hf + 1) * 512])
                ehot = a1pool.tile([4, H * 128], bf16)
                # f_src for the own slice, directly transposed: [4, SL]
                fT4 = fpsum.tile([4, SL], f32, tag="fT4", bufs=1)
                for half in range(2):
                    for c in range(2):
                        nc.tensor.matmul(
                            fT4[:, half * 512:(half + 1) * 512],
                            wf_b[:, c, W4C:W4C + 4],
                            xst_sb[:, c, half * 512:(half + 1) * 512],
                            start=(c == 0), stop=(c == 1))
                fT4_sb = a1pool.tile([4, SL], bf16)
                for hf in range(2):
                    nc.scalar.activation(
                        out=fT4_sb[:, hf * 512:(hf + 1) * 512],
                        in_=fT4[:, hf * 512:(hf + 1) * 512], func=COPY)
                # chunk-0 f_dst/q/v hoisted here: PE computes them while
                # ACT runs the U ladder below, so sweep 1 starts early.
                CB = 16
                xt0_sb = a1pool.tile([128, 2, CB * 128], bf16)
                # two halves: f_dst for blocks 0-7 (and the first q/v) can
                # start a full transfer earlier
                for hf in range(2):
                    nc.sync.dma_start(
                        xt0_sb[:, :, hf * 1024:(hf + 1) * 1024],
                        xt_d.rearrange("(c p) n -> p c n", p=128)
                            [:, :, hf * 1024:(hf + 1) * 1024])
                # one-hot rows for the per-head broadcast matmuls
                nc.sync.dma_start(ehot[:], ehot_d)
                # prefetch the first adj 4-block group ahead of chunk 1
                t = bpool.tile([128, 4, SL], bf16, tag="adjb4", bufs=2)
                nc.sync.dma_start(
                    t[:],
                    adjT_d[0:4 * 128, :]
                        .rearrange("(b p) i -> p b i", p=128))
                adjpre[0] = t
                def uladder(h):
                    fbp = fpsum.tile([128, SL], f32, tag="fbp", bufs=2)
                    for half in range(2):
                        nc.tensor.matmul(
                            fbp[:, half * 512:(half + 1) * 512],
                            ehot[:, h * 128:(h + 1) * 128],
                            fT4_sb[:, half * 512:(half + 1) * 512],
                            start=True, stop=True)
                    # U_h = exp((1-alpha) * f_src_i), straight from PSUM
                    nc.scalar.activation(
                        out=u_sb[:, h, :], in_=fbp[:],
                        func=EXP, scale=(1.0 - ALPHA))

                def qv(h, lo, hi):
                    nc.scalar.activation(
                        out=q_sb[:, h, lo:hi], in_=fall_sb[:, lo:hi, H + h],
                        func=EXP, scale=ALPHA)
                    nc.scalar.activation(
                        out=v_sb[:, h, lo:hi], in_=fall_sb[:, lo:hi, H + h],
                        func=EXP, scale=1.0)

                # heads 0-1 first (with their low-half q/v) so sweep 1's
                # first b_blocks unblock as early as possible
                uladder(0)
                uladder(1)

                def f_batch(bp, on_dve):
                    whf8e = fpsum.tile([128, 2, 8], f32, tag="whf8e", bufs=2)
                    for s in range(2):
                        for c in range(2):
                            nc.tensor.matmul(
                                whf8e[:, s, :],
                                xt0_sb[:, c, (2 * bp + s) * 128:
                                       (2 * bp + s + 1) * 128],
                                wf_b[:, c, W4C:],
                                start=(c == 0), stop=(c == 1))
                    if on_dve:
                        nc.vector.tensor_copy(
                            fall_sb[:, 2 * bp:2 * bp + 2, :], whf8e[:])
                    else:
                        nc.scalar.activation(
                            out=fall_sb[:, 2 * bp:2 * bp + 2, :],
                            in_=whf8e[:], func=COPY)

                for bp in range(CB // 2):
                    f_batch(bp, True)
                for h in (0, 1):
                    qv(h, 0, 8)
                uladder(2)
                uladder(3)
                for h in (0, 1):
                    qv(h, 8, 16)
                for h in (2, 3):
                    qv(h, 0, 16)

            # ones columns of Wh_aug (before any phase-B matmul reads them)
            for h in range(H):
                nc.vector.memset(what_sb[:, :, h * (DOUT + 1) + DOUT], 1.0)

            # --- phase B sweep over a head pair, TWO j-blocks ---
            # two blocks per iteration halve the fixed per-op costs: one
            # adj DMA, one DVE mask TT and one Pool mask TT cover 4 tiles.
            def b_block(bpool, ps2, jb0, h0, cut=None):
                # pair-sized variant for the sweep tails; shares the quad
                # tile tags (using half of each) so it costs no extra SBUF
                cut = CUT if cut is None else cut
                adj2f = bpool.tile([128, 4, SL], bf16, tag="adjb4", bufs=2)
                adj2 = adj2f[:, 0:2, :]
                nc.sync.dma_start(
                    adj2,
                    adjT_d[jb0 * 128:(jb0 + 2) * 128, :]
                        .rearrange("(b p) i -> p b i", p=128))
                t4f = bpool.tile([128, 4, 2, SL], bf16, tag="t8", bufs=2)
                t4 = t4f[:, 0:2, :, :]
                for blk in range(2):
                    for k, h in enumerate((h0, h0 + 1)):
                        nc.vector.tensor_scalar(
                            t4[:, blk, k, :], u_sb[:, h, :],
                            v_sb[:, h, jb0 + blk:jb0 + blk + 1],
                            q_sb[:, h, jb0 + blk:jb0 + blk + 1],
                            op0=MULT, op1=MAX)
                nh4f = bpool.tile([128, 4, 2, SL], bf16, tag="nh8", bufs=2)
                nh4 = nh4f[:, 0:2, :, :]
                def adj_bc(lo, hi):
                    a = adj2[:, :, lo:hi]
                    return bass.AP(tensor=a.tensor, offset=a.offset,
                                   ap=[list(a.ap[0]), list(a.ap[1]),
                                       [0, 2], [1, hi - lo]])
                nc.vector.tensor_tensor(
                    nh4[:, :, :, 0:cut], t4[:, :, :, 0:cut],
                    adj_bc(0, cut), op=MULT)
                if cut < SL:
                    nc.gpsimd.tensor_tensor(
                        nh4[:, :, :, cut:], t4[:, :, :, cut:],
                        adj_bc(cut, SL), op=MULT)
                for blk in range(2):
                    for k, h in enumerate((h0, h0 + 1)):
                        for half in range(2):
                            nc.tensor.matmul(
                                ps2[k][:, half * 512:(half + 1) * 512],
                                what_sb[:, jb0 + blk,
                                        h * (DOUT + 1):(h + 1) * (DOUT + 1)],
                                nh4[:, blk, k, half * 512:(half + 1) * 512],
                                start=(jb0 + blk == 0),
                                stop=(jb0 + blk == NBLIM - 1))

            # same as b_block but over FOUR j-blocks: one adj DMA, one DVE
            # mask op and one Pool mask op cover 8 tiles, halving the
            # per-call fixed costs (Pool Q7 launch, DVE op setup, DMA issue)
            def b_block4(bpool, ps2, jb0, h0):
                adj4 = adjpre.pop(jb0, None) if h0 == 0 else None
                if adj4 is None:
                    adj4 = bpool.tile([128, 4, SL], bf16, tag="adjb4", bufs=2)
                    nc.sync.dma_start(
                        adj4[:],
                        adjT_d[jb0 * 128:(jb0 + 4) * 128, :]
                            .rearrange("(b p) i -> p b i", p=128))
                t8 = bpool.tile([128, 4, 2, SL], bf16, tag="t8", bufs=2)
                for blk in range(4):
                    for k, h in enumerate((h0, h0 + 1)):
                        nc.vector.tensor_scalar(
                            t8[:, blk, k, :], u_sb[:, h, :],
                            v_sb[:, h, jb0 + blk:jb0 + blk + 1],
                            q_sb[:, h, jb0 + blk:jb0 + blk + 1],
                            op0=MULT, op1=MAX)
                nh8 = bpool.tile([128, 4, 2, SL], bf16, tag="nh8", bufs=2)
                def adj_bc4(lo, hi):
                    a = adj4[:, :, lo:hi]
                    return bass.AP(tensor=a.tensor, offset=a.offset,
                                   ap=[list(a.ap[0]), list(a.ap[1]),
                                       [0, 2], [1, hi - lo]])
                nc.vector.tensor_tensor(
                    nh8[:, :, :, 0:CUT], t8[:, :, :, 0:CUT],
                    adj_bc4(0, CUT), op=MULT)
                nc.gpsimd.tensor_tensor(
                    nh8[:, :, :, CUT:], t8[:, :, :, CUT:],
                    adj_bc4(CUT, SL), op=MULT)
                for blk in range(4):
                    for k, h in enumerate((h0, h0 + 1)):
                        for half in range(2):
                            nc.tensor.matmul(
                                ps2[k][:, half * 512:(half + 1) * 512],
                                what_sb[:, jb0 + blk,
                                        h * (DOUT + 1):(h + 1) * (DOUT + 1)],
                                nh8[:, blk, k, half * 512:(half + 1) * 512],
                                start=(jb0 + blk == 0),
                                stop=(jb0 + blk == NBLIM - 1))

            # --- xT -> f_dst/q/v then Wh per n-block; sweep 1 lags one
            # chunk so `what` columns land before their psum matmuls ---
            CB = 16
            with ExitStack() as sctx:
                apool = sctx.enter_context(tc.tile_pool(name="aphase", bufs=2))
                apsum = sctx.enter_context(
                    tc.tile_pool(name="apsum_x", bufs=2, space="PSUM"))
                fpsum2 = sctx.enter_context(
                    tc.tile_pool(name="apsum_f2", bufs=2, space="PSUM"))
                bpsum1 = sctx.enter_context(
                    tc.tile_pool(name="bpsum1", bufs=1, space="PSUM"))
                ps01 = [bpsum1.tile([DOUT + 1, SL], f32, tag=f"acc{h}",
                                    name=f"acc{h}") for h in range(2)]
                for cb0 in range(0, NB, CB):
                    if cb0 == 0:
                        xt_chunk = xt0_sb
                    else:
                        xt_chunk = apool.tile([128, 2, CB * 128], bf16,
                                              tag="xtchunk")
                        nc.sync.dma_start(
                            xt_chunk[:],
                            xt_d.rearrange("(c p) n -> p c n", p=128)
                                [:, :, cb0 * 128:(cb0 + CB) * 128])
                    # f_dst columns first (tiny matmuls) -> q, v
                    # (chunk 0 was hoisted into the U section above)
                    for bp in (range(CB // 2) if cb0 > 0 else ()):
                        whf8 = fpsum2.tile([128, 2, 8], f32, tag="whf8")
                        for s in range(2):
                            bi = 2 * bp + s
                            for c in range(2):
                                nc.tensor.matmul(
                                    whf8[:, s, :],
                                    xt_chunk[:, c, bi * 128:(bi + 1) * 128],
                                    wf_b[:, c, W4C:],
                                    start=(c == 0), stop=(c == 1))
                        nc.scalar.activation(
                            out=fall_sb[:, cb0 + 2 * bp:cb0 + 2 * bp + 2, :],
                            in_=whf8[:], func=COPY)
                    for h in (range(H) if cb0 > 0 else ()):
                        nc.scalar.activation(
                            out=q_sb[:, h, cb0:cb0 + CB],
                            in_=fall_sb[:, cb0:cb0 + CB, H + h],
                            func=EXP, scale=ALPHA)
                        nc.scalar.activation(
                            out=v_sb[:, h, cb0:cb0 + CB],
                            in_=fall_sb[:, cb0:cb0 + CB, H + h],
                            func=EXP, scale=1.0)
                    # Wh columns, interleaved 1:1 with sweep-1 blocks of
                    # the PREVIOUS chunk so PE never sees a long A2 burst
                    for bi in range(CB):
                        b = cb0 + bi
                        whf = apsum.tile([128, W4C], f32, tag="whf")
                        for c in range(2):
                            nc.tensor.matmul(
                                whf[:],
                                xt_chunk[:, c, bi * 128:(bi + 1) * 128],
                                wf_b[:, c, 0:W4C],
                                start=(c == 0), stop=(c == 1))
                        nc.scalar.activation(
                            out=what_sb[:, b, :]
                                .rearrange("p (h o) -> p h o", h=H)[:, :, 0:DOUT],
                            in_=whf[:].rearrange("p (h o) -> p h o", h=H),
                            func=COPY)
                        if cb0 > 0 and bi % 4 == 0 and cb0 - CB + bi < NBLIM:
                            b_block4(bpool, ps01, cb0 - CB + bi, 0)
                for jb in range(NB - CB, min(NB, NBLIM), 4):
                    b_block4(bpool, ps01, jb, 0)
                for h in range(2):
                    nc.scalar.activation(out=ps_sb[:, h, :], in_=ps01[h][:],
                                         func=COPY)

        # ============ PHASE B sweep 2 + PHASE C per head pair ============
        with ExitStack() as tctx:
            c2pool = tctx.enter_context(tc.tile_pool(name="c2", bufs=4))
            cpsum = tctx.enter_context(
                tc.tile_pool(name="cpsum", bufs=4, space="PSUM"))

            def c_bi(h0, bi, o_all):
                # output rows bi*128.. for heads h0, h0+1 into pair staging
                for k, h in enumerate((h0, h0 + 1)):
                    pst = cpsum.tile([128, DOUT + 1], f32, tag="pst")
                    nc.tensor.transpose(
                        pst[:], ps_sb[:, h, bi * 128:(bi + 1) * 128],
                        ident_sb[0:DOUT + 1, 0:DOUT + 1])
                    rec = c2pool.tile([128, 1], f32, tag="rec")
                    nc.vector.reciprocal(rec[:], pst[:, DOUT:DOUT + 1])
                    nc.scalar.activation(
                        out=o_all[:, bi, k * DOUT:(k + 1) * DOUT],
                        in_=pst[:, 0:DOUT], func=COPY, scale=rec[:])

            def c_flush(h0, o_all):
                # single DMA for the whole pair: rows grouped per 128-block
                nc.sync.dma_start(
                    out_d.rearrange("(b p) c -> p b c", p=128)
                         [:, :, h0 * DOUT:(h0 + 2) * DOUT], o_all[:])

            with ExitStack() as bctx:
                bpsum2 = bctx.enter_context(
                    tc.tile_pool(name="bpsum2", bufs=1, space="PSUM"))
                ps23 = [bpsum2.tile([DOUT + 1, SL], f32, tag=f"acc{h+2}",
                                    name=f"acc{h+2}") for h in range(2)]
                o_all0 = c2pool.tile([128, SB, 2 * DOUT], bf16, tag="oall0")
                for jb in range(0, NBLIM - 4, 4):
                    b_block4(bpool, ps23, jb, 2)
                    # pair-0 epilogue interleaved so its DVE/PE ops never
                    # head-block sweep 2's queue
                    if jb % 8 == 4:
                        c_bi(0, jb // 8, o_all0)
                b_block(bpool, ps23, NBLIM - 4, 2)
                c_bi(0, 7, o_all0)
                b_block(bpool, ps23, NBLIM - 2, 2, cut=SL)
                c_flush(0, o_all0)
                for h in range(2):
                    for hf in range(2):
                        nc.scalar.activation(
                            out=ps_sb[:, h + 2, hf * 512:(hf + 1) * 512],
                            in_=ps23[h][:, hf * 512:(hf + 1) * 512],
                            func=COPY)
            # pair 2-3 epilogue, head-at-a-time so head 2's transposes
            # start right after its own staging copy
            o_all2 = c2pool.tile([128, SB, 2 * DOUT], bf16, tag="oall2")
            for k in range(2):
                for bi in range(SB):
                    pst = cpsum.tile([128, DOUT + 1], f32, tag="pst")
                    nc.tensor.transpose(
                        pst[:], ps_sb[:, 2 + k, bi * 128:(bi + 1) * 128],
                        ident_sb[0:DOUT + 1, 0:DOUT + 1])
                    rec = c2pool.tile([128, 1], f32, tag="rec")
                    nc.vector.reciprocal(rec[:], pst[:, DOUT:DOUT + 1])
                    nc.scalar.activation(
                        out=o_all2[:, bi, k * DOUT:(k + 1) * DOUT],
                        in_=pst[:, 0:DOUT], func=COPY, scale=rec[:])
                    # flush early halves so only a short DMA trails the
                    # last scale of the final head
                    if k == 1 and bi == SB // 2 - 1:
                        nc.sync.dma_start(
                            out_d.rearrange("(b p) c -> p b c", p=128)
                                 [:, 0:SB // 2, 3 * DOUT:],
                            o_all2[:, 0:SB // 2, DOUT:])
                # flush this head's 64 columns while the other head runs
                if k == 0:
                    nc.sync.dma_start(
                        out_d.rearrange("(b p) c -> p b c", p=128)
                             [:, :, 2 * DOUT:3 * DOUT],
                        o_all2[:, :, 0:DOUT])
                else:
                    nc.sync.dma_start(
                        out_d.rearrange("(b p) c -> p b c", p=128)
                             [:, SB // 2:, 3 * DOUT:],
                        o_all2[:, SB // 2:, DOUT:])

    nc.compile()
    return nc


def kernel(x, adj, W, a_src, a_dst):
    import ml_dtypes
    x = np.asarray(x, dtype=np.float32)
    adj = np.asarray(adj)
    W = np.ascontiguousarray(np.asarray(W, dtype=np.float32))
    a_all = np.ascontiguousarray(
        np.stack([np.asarray(a_src, np.float32),
                  np.asarray(a_dst, np.float32)], axis=1))  # [H, 2, DOUT]
    # fused weights: [W4 | wtilde] with wtilde = W @ a (weight-only prep)
    wt = np.einsum('hdo,hso->hds', W, a_all)          # [H, DIN, 2]
    wf = np.zeros((128, 2, W4C + 8), dtype=np.float32)
    for h in range(H):
        for c in range(2):
            wf[:, c, h * DOUT:(h + 1) * DOUT] = W[h, c * 128:(c + 1) * 128, :]
            for s in range(2):
                wf[:, c, W4C + s * 4 + h] = wt[h, c * 128:(c + 1) * 128, s]
    wf_bf16 = wf.astype(ml_dtypes.bfloat16)
    ehot = np.zeros((4, H * 128), dtype=ml_dtypes.bfloat16)
    for h in range(H):
        ehot[h, h * 128:(h + 1) * 128] = 1.0
    # bf16 cast of the 0/1 mask is exact
    adjT_bf16 = np.ascontiguousarray(adj.T).astype(ml_dtypes.bfloat16)
    xT_bf16 = np.ascontiguousarray(x.T.astype(ml_dtypes.bfloat16))

    if "nc" not in _CACHE:
        _CACHE["nc"] = _build_module()
    nc = _CACHE["nc"]

    in_maps = []
    for c in range(NCORES):
        sl = slice(c * SL, (c + 1) * SL)
        in_maps.append({
            "xT_full": xT_bf16,
            "xT_slice": np.ascontiguousarray(xT_bf16[:, sl]),
            "wf_all": wf_bf16,
            "ehot_const": ehot,
            "adjT_slice": np.ascontiguousarray(adjT_bf16[:, sl]),
        })
    res = run_bass_kernel_spmd(nc, in_maps, core_ids=list(range(NCORES)))
    out = np.concatenate([res.results[c]["out_slice"] for c in range(NCORES)],
                         axis=0)
    return out.astype(np.float32)



# revision 48
# speedup vs baseline: 1.0251x; 1.0251x over previous
"""GAT (graph attention) layer on 8 Trainium2 NeuronCores.

Reference math (per head h):
    Wh = x @ W[h];  f_src = Wh @ a_src[h];  f_dst = Wh @ a_dst[h]
    e[i,j] = leaky_relu(f_src[i] + f_dst[j], alpha)
    att = softmax(where(adj>0, e, -9e15), axis=j)
    out[:, h*D:(h+1)*D] = att @ Wh

Exact identity used (p_i = exp(alpha*f_src_i) cancels in the softmax):
    exp(leaky_relu(s)) = exp(alpha*s) * max(1, exp((1-alpha)*s))
    nhat[j,i] = adj[j,i] * max(q_j, u_i * v_j)
      with u_i = exp((1-alpha)*f_src_i), v_j = exp(f_dst_j),
           q_j = exp(alpha*f_dst_j)
    out_h[i,:] = (sum_j nhat[j,i]*Wh[j,:]) / (sum_j nhat[j,i])

Inner loop per (j-block, head-pair): two dual-scalar tensor_scalar ops
    t_h = max(U_h * v_j, q_j)        (DVE 4x mode, per-partition scalars)
then the mask multiply t_h * adj is split by columns between DVE
(tensor_tensor at 2x) and Pool (tensor_tensor, runs in parallel), and
accumulated into PSUM by the usual matmuls.

Phase B runs as two head-pair sweeps of 4 PSUM banks each so sweep 1
overlaps phase A's Wh production (which needs the other banks), and
each pair's phase C overlaps the other sweep. DMA issue order keeps
the small weight transfers ahead of the bulk xT streams; q/v/U are
produced early so the sweeps start as soon as possible.

Distribution: destination rows i sharded 1024/core; host passes adj.T
column-slices cast to bf16 (exact for a 0/1 mask) and x pre-transposed
in bf16, so no device-side transposes or converts are needed.
"""

import os
import numpy as np
from contextlib import ExitStack

import concourse.bass as bass
import concourse.tile as tile
from concourse import bacc, mybir
from concourse.bass_utils import run_bass_kernel_spmd
from concourse.masks import make_identity

N = 8192
DIN = 256
DOUT = 64
H = 4
NCORES = 8
SL = N // NCORES          # 1024 i's per core
NB = N // 128             # 64 j-blocks
SB = SL // 128            # 8 i-blocks per core
ALPHA = 0.2
W4C = H * DOUT            # 256
WAUG = H * (DOUT + 1)     # 260

f32 = mybir.dt.float32
bf16 = mybir.dt.bfloat16
EXP = mybir.ActivationFunctionType.Exp
COPY = mybir.ActivationFunctionType.Copy
MAX = mybir.AluOpType.max
MULT = mybir.AluOpType.mult

_CACHE = {}

NBLIM = int(os.environ.get("NBLIM", str(NB)))
BBUFS = int(os.environ.get("BBUFS", "4"))
CUT = int(os.environ.get("CUT", "684"))     # DVE/Pool mask column split


def _build_module():
    nc = bacc.Bacc("TRN2", target_bir_lowering=False, debug=False,
                   num_devices=NCORES)

    xt_d = nc.dram_tensor("xT_full", [DIN, N], bf16, kind="ExternalInput").ap()
    xst_d = nc.dram_tensor("xT_slice", [DIN, SL], bf16, kind="ExternalInput").ap()
    wf_d = nc.dram_tensor("wf_all", [128, 2, W4C + 8], bf16,
                          kind="ExternalInput").ap()
    ehot_d = nc.dram_tensor("ehot_const", [4, H * 128], bf16,
                            kind="ExternalInput").ap()
    adjT_d = nc.dram_tensor("adjT_slice", [N, SL], bf16, kind="ExternalInput").ap()
    out_d = nc.dram_tensor("out_slice", [SL, H * DOUT], bf16,
                       kind="ExternalOutput").ap()

    with tile.TileContext(nc) as tc, ExitStack() as ctx:
        # ---------------- persistent tiles ----------------
        persist = ctx.enter_context(tc.tile_pool(name="persist", bufs=1))
        what_sb = persist.tile([128, NB, WAUG], bf16)   # Wh_aug per j-block
        u_sb = persist.tile([128, H, SL], bf16)         # exp((1-a)*f_src) bcast
        fall_sb = persist.tile([128, NB, 2 * H], f32)   # [f_src(4) | f_dst(4)]
        q_sb = persist.tile([128, H, NB], f32)          # exp(alpha*f_dst)
        v_sb = persist.tile([128, H, NB], f32)          # exp(f_dst)
        ps_sb = persist.tile([DOUT + 1, H, SL], f32)    # phase C staging
        ident_sb = persist.tile([128, 128], f32)
        make_identity(nc, ident_sb[:])
        bpool = ctx.enter_context(tc.tile_pool(name="bphase", bufs=BBUFS))
        adjpre = {}

        # ======================= PHASE A =======================
        with ExitStack() as actx:
            a1pool = actx.enter_context(tc.tile_pool(name="aphase1", bufs=1))

            # --- fused weight matrix [W4 | wtilde] (host-prepared) ---
            wf_b = a1pool.tile([128, 2, W4C + 8], bf16)
            nc.sync.dma_start(wf_b[:], wf_d)

            # --- own-slice f_src -> broadcast -> U  (early: unblocks B) ---
            with ExitStack() as sctx:
                fpsum = sctx.enter_context(
                    tc.tile_pool(name="apsum_f", bufs=3, space="PSUM"))
                xst_sb = a1pool.tile([128, 2, SL], bf16)
                for hf in range(2):
                    nc.sync.dma_start(
                        xst_sb[:, :, hf * 512:(hf + 1) * 512],
                        xst_d.rearrange("(c p) n -> p c n", p=128)
                            [:, :, hf * 512:(hf + 1) * 512])
                ehot = a1pool.tile([4, H * 128], bf16)
                # f_src for the own slice, directly transposed: [4, SL]
                fT4 = fpsum.tile([4, SL], f32, tag="fT4", bufs=1)
                for half in range(2):
                    for c in range(2):
                        nc.tensor.matmul(
                            fT4[:, half * 512:(half + 1) * 512],
                            wf_b[:, c, W4C:W4C + 4],
                            xst_sb[:, c, half * 512:(half + 1) * 512],
                            start=(c == 0), stop=(c == 1))
                fT4_sb = a1pool.tile([4, SL], bf16)
                for hf in range(2):
                    nc.scalar.activation(
                        out=fT4_sb[:, hf * 512:(hf + 1) * 512],
                        in_=fT4[:, hf * 512:(hf + 1) * 512], func=COPY)
                # chunk-0 f_dst/q/v hoisted here: PE computes them while
                # ACT runs the U ladder below, so sweep 1 starts early.
                CB = 16
                xt0_sb = a1pool.tile([128, 2, CB * 128], bf16)
                # two halves: f_dst for blocks 0-7 (and the first q/v) can
                # start a full transfer earlier
                for hf in range(2):
                    nc.sync.dma_start(
                        xt0_sb[:, :, hf * 1024:(hf + 1) * 1024],
                        xt_d.rearrange("(c p) n -> p c n", p=128)
                            [:, :, hf * 1024:(hf + 1) * 1024])
                # one-hot rows for the per-head broadcast matmuls
                nc.sync.dma_start(ehot[:], ehot_d)
                # prefetch the first two adj block-pairs ahead of chunk 1
                for jb0 in (0, 2):
                    t = bpool.tile([128, 2, SL], bf16, tag="adjb")
                    nc.sync.dma_start(
                        t[:],
                        adjT_d[jb0 * 128:(jb0 + 2) * 128, :]
                            .rearrange("(b p) i -> p b i", p=128))
                    adjpre[jb0] = t
                def uladder(h):
                    fbp = fpsum.tile([128, SL], f32, tag="fbp", bufs=2)
                    for half in range(2):
                        nc.tensor.matmul(
                            fbp[:, half * 512:(half + 1) * 512],
                            ehot[:, h * 128:(h + 1) * 128],
                            fT4_sb[:, half * 512:(half + 1) * 512],
                            start=True, stop=True)
                    # U_h = exp((1-alpha) * f_src_i), straight from PSUM
                    nc.scalar.activation(
                        out=u_sb[:, h, :], in_=fbp[:],
                        func=EXP, scale=(1.0 - ALPHA))

                def qv(h, lo, hi):
                    nc.scalar.activation(
                        out=q_sb[:, h, lo:hi], in_=fall_sb[:, lo:hi, H + h],
                        func=EXP, scale=ALPHA)
                    nc.scalar.activation(
                        out=v_sb[:, h, lo:hi], in_=fall_sb[:, lo:hi, H + h],
                        func=EXP, scale=1.0)

                # heads 0-1 first (with their low-half q/v) so sweep 1's
                # first b_blocks unblock as early as possible
                uladder(0)
                uladder(1)

                def f_batch(bp, on_dve):
                    whf8e = fpsum.tile([128, 2, 8], f32, tag="whf8e", bufs=2)
                    for s in range(2):
                        for c in range(2):
                            nc.tensor.matmul(
                                whf8e[:, s, :],
                                xt0_sb[:, c, (2 * bp + s) * 128:
                                       (2 * bp + s + 1) * 128],
                                wf_b[:, c, W4C:],
                                start=(c == 0), stop=(c == 1))
                    if on_dve:
                        nc.vector.tensor_copy(
                            fall_sb[:, 2 * bp:2 * bp + 2, :], whf8e[:])
                    else:
                        nc.scalar.activation(
                            out=fall_sb[:, 2 * bp:2 * bp + 2, :],
                            in_=whf8e[:], func=COPY)

                for bp in range(CB // 2):
                    f_batch(bp, True)
                for h in (0, 1):
                    qv(h, 0, 8)
                uladder(2)
                uladder(3)
                for h in (0, 1):
                    qv(h, 8, 16)
                for h in (2, 3):
                    qv(h, 0, 16)

            # ones columns of Wh_aug (before any phase-B matmul reads them)
            for h in range(H):
                nc.vector.memset(what_sb[:, :, h * (DOUT + 1) + DOUT], 1.0)

            # --- phase B sweep over a head pair, TWO j-blocks ---
            # two blocks per iteration halve the fixed per-op costs: one
            # adj DMA, one DVE mask TT and one Pool mask TT cover 4 tiles.
            def b_block(bpool, ps2, jb0, h0, cut=None):
                cut = CUT if cut is None else cut
                adj2 = adjpre.pop(jb0, None) if h0 == 0 else None
                if adj2 is None:
                    adj2 = bpool.tile([128, 2, SL], bf16, tag="adjb")
                    nc.sync.dma_start(
                        adj2[:],
                        adjT_d[jb0 * 128:(jb0 + 2) * 128, :]
                            .rearrange("(b p) i -> p b i", p=128))
                t4 = bpool.tile([128, 2, 2, SL], bf16, tag="t2")
                for blk in range(2):
                    for k, h in enumerate((h0, h0 + 1)):
                        nc.vector.tensor_scalar(
                            t4[:, blk, k, :], u_sb[:, h, :],
                            v_sb[:, h, jb0 + blk:jb0 + blk + 1],
                            q_sb[:, h, jb0 + blk:jb0 + blk + 1],
                            op0=MULT, op1=MAX)
                nh4 = bpool.tile([128, 2, 2, SL], bf16, tag="nh2")
                def adj_bc(lo, hi):
                    a = adj2[:, :, lo:hi]
                    return bass.AP(tensor=a.tensor, offset=a.offset,
                                   ap=[list(a.ap[0]), list(a.ap[1]),
                                       [0, 2], [1, hi - lo]])
                nc.vector.tensor_tensor(
                    nh4[:, :, :, 0:cut], t4[:, :, :, 0:cut],
                    adj_bc(0, cut), op=MULT)
                if cut < SL:
                    nc.gpsimd.tensor_tensor(
                        nh4[:, :, :, cut:], t4[:, :, :, cut:],
                        adj_bc(cut, SL), op=MULT)
                for blk in range(2):
                    for k, h in enumerate((h0, h0 + 1)):
                        for half in range(2):
                            nc.tensor.matmul(
                                ps2[k][:, half * 512:(half + 1) * 512],
                                what_sb[:, jb0 + blk,
                                        h * (DOUT + 1):(h + 1) * (DOUT + 1)],
                                nh4[:, blk, k, half * 512:(half + 1) * 512],
                                start=(jb0 + blk == 0),
                                stop=(jb0 + blk == NBLIM - 1))

            # --- xT -> f_dst/q/v then Wh per n-block; sweep 1 lags one
            # chunk so `what` columns land before their psum matmuls ---
            CB = 16
            with ExitStack() as sctx:
                apool = sctx.enter_context(tc.tile_pool(name="aphase", bufs=2))
                apsum = sctx.enter_context(
                    tc.tile_pool(name="apsum_x", bufs=2, space="PSUM"))
                fpsum2 = sctx.enter_context(
                    tc.tile_pool(name="apsum_f2", bufs=2, space="PSUM"))
                bpsum1 = sctx.enter_context(
                    tc.tile_pool(name="bpsum1", bufs=1, space="PSUM"))
                ps01 = [bpsum1.tile([DOUT + 1, SL], f32, tag=f"acc{h}",
                                    name=f"acc{h}") for h in range(2)]
                for cb0 in range(0, NB, CB):
                    if cb0 == 0:
                        xt_chunk = xt0_sb
                    else:
                        xt_chunk = apool.tile([128, 2, CB * 128], bf16,
                                              tag="xtchunk")
                        nc.sync.dma_start(
                            xt_chunk[:],
                            xt_d.rearrange("(c p) n -> p c n", p=128)
                                [:, :, cb0 * 128:(cb0 + CB) * 128])
                    # f_dst columns first (tiny matmuls) -> q, v
                    # (chunk 0 was hoisted into the U section above)
                    for bp in (range(CB // 2) if cb0 > 0 else ()):
                        whf8 = fpsum2.tile([128, 2, 8], f32, tag="whf8")
                        for s in range(2):
                            bi = 2 * bp + s
                            for c in range(2):
                                nc.tensor.matmul(
                                    whf8[:, s, :],
                                    xt_chunk[:, c, bi * 128:(bi + 1) * 128],
                                    wf_b[:, c, W4C:],
                                    start=(c == 0), stop=(c == 1))
                        nc.scalar.activation(
                            out=fall_sb[:, cb0 + 2 * bp:cb0 + 2 * bp + 2, :],
                            in_=whf8[:], func=COPY)
                    for h in (range(H) if cb0 > 0 else ()):
                        nc.scalar.activation(
                            out=q_sb[:, h, cb0:cb0 + CB],
                            in_=fall_sb[:, cb0:cb0 + CB, H + h],
                            func=EXP, scale=ALPHA)
                        nc.scalar.activation(
                            out=v_sb[:, h, cb0:cb0 + CB],
                            in_=fall_sb[:, cb0:cb0 + CB, H + h],
                            func=EXP, scale=1.0)
                    # Wh columns, interleaved 1:1 with sweep-1 blocks of
                    # the PREVIOUS chunk so PE never sees a long A2 burst
                    for bi in range(CB):
                        b = cb0 + bi
                        whf = apsum.tile([128, W4C], f32, tag="whf")
                        for c in range(2):
                            nc.tensor.matmul(
                                whf[:],
                                xt_chunk[:, c, bi * 128:(bi + 1) * 128],
                                wf_b[:, c, 0:W4C],
                                start=(c == 0), stop=(c == 1))
                        nc.scalar.activation(
                            out=what_sb[:, b, :]
                                .rearrange("p (h o) -> p h o", h=H)[:, :, 0:DOUT],
                            in_=whf[:].rearrange("p (h o) -> p h o", h=H),
                            func=COPY)
                        if cb0 > 0 and bi % 2 == 0 and cb0 - CB + bi < NBLIM:
                            b_block(bpool, ps01, cb0 - CB + bi, 0)
                for jb in range(NB - CB, min(NB, NBLIM), 2):
                    b_block(bpool, ps01, jb, 0)
                for h in range(2):
                    nc.scalar.activation(out=ps_sb[:, h, :], in_=ps01[h][:],
                                         func=COPY)

        # ============ PHASE B sweep 2 + PHASE C per head pair ============
        with ExitStack() as tctx:
            c2pool = tctx.enter_context(tc.tile_pool(name="c2", bufs=4))
            cpsum = tctx.enter_context(
                tc.tile_pool(name="cpsum", bufs=4, space="PSUM"))

            def c_bi(h0, bi, o_all):
                # output rows bi*128.. for heads h0, h0+1 into pair staging
                for k, h in enumerate((h0, h0 + 1)):
                    pst = cpsum.tile([128, DOUT + 1], f32, tag="pst")
                    nc.tensor.transpose(
                        pst[:], ps_sb[:, h, bi * 128:(bi + 1) * 128],
                        ident_sb[0:DOUT + 1, 0:DOUT + 1])
                    rec = c2pool.tile([128, 1], f32, tag="rec")
                    nc.vector.reciprocal(rec[:], pst[:, DOUT:DOUT + 1])
                    nc.scalar.activation(
                        out=o_all[:, bi, k * DOUT:(k + 1) * DOUT],
                        in_=pst[:, 0:DOUT], func=COPY, scale=rec[:])

            def c_flush(h0, o_all):
                # single DMA for the whole pair: rows grouped per 128-block
                nc.sync.dma_start(
                    out_d.rearrange("(b p) c -> p b c", p=128)
                         [:, :, h0 * DOUT:(h0 + 2) * DOUT], o_all[:])

            with ExitStack() as bctx:
                bpsum2 = bctx.enter_context(
                    tc.tile_pool(name="bpsum2", bufs=1, space="PSUM"))
                ps23 = [bpsum2.tile([DOUT + 1, SL], f32, tag=f"acc{h+2}",
                                    name=f"acc{h+2}") for h in range(2)]
                o_all0 = c2pool.tile([128, SB, 2 * DOUT], bf16, tag="oall0")
                for jb in range(0, NBLIM, 2):
                    b_block(bpool, ps23, jb, 2,
                            cut=(SL if jb >= NBLIM - 2 else None))
                    # pair-0 epilogue interleaved so its DVE/PE ops never
                    # head-block sweep 2's queue
                    if jb % 8 == 6:
                        c_bi(0, jb // 8, o_all0)
                c_flush(0, o_all0)
                for h in range(2):
                    for hf in range(2):
                        nc.scalar.activation(
                            out=ps_sb[:, h + 2, hf * 512:(hf + 1) * 512],
                            in_=ps23[h][:, hf * 512:(hf + 1) * 512],
                            func=COPY)
            # pair 2-3 epilogue, head-at-a-time so head 2's transposes
            # start right after its own staging copy
            o_all2 = c2pool.tile([128, SB, 2 * DOUT], bf16, tag="oall2")
            for k in range(2):
                for bi in range(SB):
                    pst = cpsum.tile([128, DOUT + 1], f32, tag="pst")
                    nc.tensor.transpose(
                        pst[:], ps_sb[:, 2 + k, bi * 128:(bi + 1) * 128],
                        ident_sb[0:DOUT + 1, 0:DOUT + 1])
                    rec = c2pool.tile([128, 1], f32, tag="rec")
                    nc.vector.reciprocal(rec[:], pst[:, DOUT:DOUT + 1])
                    nc.scalar.activation(
                        out=o_all2[:, bi, k * DOUT:(k + 1) * DOUT],
                        in_=pst[:, 0:DOUT], func=COPY, scale=rec[:])
                    # flush early halves so only a short DMA trails the
                    # last scale of the final head
                    if k == 1 and bi == SB // 2 - 1:
                        nc.sync.dma_start(
                            out_d.rearrange("(b p) c -> p b c", p=128)
                                 [:, 0:SB // 2, 3 * DOUT:],
                            o_all2[:, 0:SB // 2, DOUT:])
                # flush this head's 64 columns while the other head runs
                if k == 0:
                    nc.sync.dma_start(
                        out_d.rearrange("(b p) c -> p b c", p=128)
                             [:, :, 2 * DOUT:3 * DOUT],
                        o_all2[:, :, 0:DOUT])
                else:
                    nc.sync.dma_start(
                        out_d.rearrange("(b p) c -> p b c", p=128)
                             [:, SB // 2:, 3 * DOUT:],
                        o_all2[:, SB // 2:, DOUT:])

    nc.compile()
    return nc


def kernel(x, adj, W, a_src, a_dst):
    import ml_dtypes
    x = np.asarray(x, dtype=np.float32)
    adj = np.asarray(adj)
    W = np.ascontiguousarray(np.asarray(W, dtype=np.float32))
    a_all = np.ascontiguousarray(
        np.stack([np.asarray(a_src, np.float32),
                  np.asarray(a_dst, np.float32)], axis=1))  # [H, 2, DOUT]
    # fused weights: [W4 | wtilde] with wtilde = W @ a (weight-only prep)
    wt = np.einsum('hdo,hso->hds', W, a_all)          # [H, DIN, 2]
    wf = np.zeros((128, 2, W4C + 8), dtype=np.float32)
    for h in range(H):
        for c in range(2):
            wf[:, c, h * DOUT:(h + 1) * DOUT] = W[h, c * 128:(c + 1) * 128, :]
            for s in range(2):
                wf[:, c, W4C + s * 4 + h] = wt[h, c * 128:(c + 1) * 128, s]
    wf_bf16 = wf.astype(ml_dtypes.bfloat16)
    ehot = np.zeros((4, H * 128), dtype=ml_dtypes.bfloat16)
    for h in range(H):
        ehot[h, h * 128:(h + 1) * 128] = 1.0
    # bf16 cast of the 0/1 mask is exact
    adjT_bf16 = np.ascontiguousarray(adj.T).astype(ml_dtypes.bfloat16)
    xT_bf16 = np.ascontiguousarray(x.T.astype(ml_dtypes.bfloat16))

    if "nc" not in _CACHE:
        _CACHE["nc"] = _build_module()
    nc = _CACHE["nc"]

    in_maps = []
    for c in range(NCORES):
        sl = slice(c * SL, (c + 1) * SL)
        in_maps.append({
            "xT_full": xT_bf16,
            "xT_slice": np.ascontiguousarray(xT_bf16[:, sl]),
            "wf_all": wf_bf16,
            "ehot_const": ehot,
            "adjT_slice": np.ascontiguousarray(adjT_bf16[:, sl]),
        })
    res = run_bass_kernel_spmd(nc, in_maps, core_ids=list(range(NCORES)))
    out = np.concatenate([res.results[c]["out_slice"] for c in range(NCORES)],
                         axis=0)
    return out.astype(np.float32)

